# revision 1
# baseline (speedup 1.0000x reference)
"""Trainium2 Bass kernel for nn_ControlledConvEMAStabilizer.

Pipeline (per batch image, one NeuronCore each, batch-parallel over 8 cores):
  q = cat(backbone, z, mem_stab, mem_unstab)          # 160ch
  q = lrelu(conv3x3(q, w0) + b0)                      # -> 64ch
  q = lrelu(conv3x3(q, w1) + b1)                      # -> 64ch
  q = lrelu(conv3x3(q, w2) + b2)                      # -> 64ch
  head = conv3x3(q, w_last) + b_last                  # -> 288ch = 9 taps x 32ch
  eta  = softmax([head; 0]) over the 9+1 slots
  out  = sum_p unfold(mem_stab)[p] * eta[p] + eta[9] * z

Implementation notes:
  - Feature maps live in SBUF as zero-padded flat rows: image pixel (r,c) at
    column 129*(r+1)+1+c  (row stride 129, shared single pad column between
    rows, one pad row top/bottom).  Every 3x3 tap is then a pure column
    offset t = 129*dr + dc, so convs are PSUM-accumulated matmuls over
    shifted views (float32r -> full PE rate at N>=256).
  - K-stacking: each intermediate tensor is stored twice in one [128, NCOL]
    tile: partitions 0:64 = q, partitions 64:128 = q shifted by +129 (one
    image row).  A K=128 matmul then applies two vertical taps at once.
  - LeakyReLU: y = (x + b) + Relu(-0.99*(x + b)), via one ScalarE activation
    (scale=-0.99, bias=-0.99b) + one fused DVE scalar_tensor_tensor.
  - Tail fused per 3-row strip: conv_last (18 mm) -> Exp(+b_last) on ACT ->
    multiply with shifted mem_stab patches (DVE) -> partition-group sums via
    block-identity matmuls (PE) -> reciprocal_approx_fast -> out.
"""

import numpy as np
from contextlib import ExitStack

import concourse.bacc as bacc
import concourse.tile as tile
from concourse import mybir
from concourse.bass_utils import run_bass_kernel_spmd

F32 = mybir.dt.float32
F32R = mybir.dt.float32r
BF16 = mybir.dt.bfloat16
ALU = mybir.AluOpType
ACTF = mybir.ActivationFunctionType

H = 128
ST = 129                      # padded row stride
NCOL = ST * 130 + 2           # 16772 sbuf cols (incl 1 extra tail zero)
XCOL = NCOL                   # dram padded cols for xpad
MUCOL = NCOL + 2 * ST + 2     # mu needs reads up to +258 further
ROWS_PER_STRIP = 3
X_GROUP_STRIPS = 3            # conv0 input staging granularity (9 rows)

# taps in fusion/unfold order p = 3*kh + kw -> offset 129*(kh-1) + (kw-1)
P_TAPS = [ST * (kh - 1) + (kw - 1) for kh in range(3) for kw in range(3)]


def _j0(r0):
    return ST * (r0 + 1) + 1


def _strips():
    out = []
    r0 = 0
    while r0 < H:
        nr = min(ROWS_PER_STRIP, H - r0)
        out.append((r0, nr))
        r0 += nr
    return out


def _build_program(debug=False):
    nc = bacc.Bacc("TRN2", target_bir_lowering=False, debug=False)

    d_xpad = nc.dram_tensor("xpad", [128, XCOL], BF16, kind="ExternalInput")
    d_mupad = nc.dram_tensor("mupad", [32, MUCOL], BF16, kind="ExternalInput")
    d_w0c1 = nc.dram_tensor("w0c1", [128, 9 * 64], BF16, kind="ExternalInput")
    d_w0c2 = nc.dram_tensor("w0c2", [96, 3 * 64], BF16, kind="ExternalInput")
    d_w1P = nc.dram_tensor("w1P", [128, 3 * 64], BF16, kind="ExternalInput")
    d_w1S = nc.dram_tensor("w1S", [64, 3 * 64], BF16, kind="ExternalInput")
    d_w2P = nc.dram_tensor("w2P", [128, 3 * 64], BF16, kind="ExternalInput")
    d_w2S = nc.dram_tensor("w2S", [64, 3 * 64], BF16, kind="ExternalInput")
    d_wlP = nc.dram_tensor("wlP", [128, 3 * 288], BF16, kind="ExternalInput")
    d_wlS = nc.dram_tensor("wlS", [64, 3 * 288], BF16, kind="ExternalInput")
    d_b = nc.dram_tensor("bias", [64, 6], F32, kind="ExternalInput")  # b0,b0n,b1,b1n,b2,b2n
    d_blp = nc.dram_tensor("blp", [128, 3], F32, kind="ExternalInput")  # 288 perm bias, col-chunks
    d_eye = nc.dram_tensor("eye", [128, 32], BF16, kind="ExternalInput")
    d_out = nc.dram_tensor("out", [32, H, H], F32, kind="ExternalOutput")
    if debug:
        d_q1 = nc.dram_tensor("dbg_q1", [128, NCOL], F32, kind="ExternalOutput")
        d_q2 = nc.dram_tensor("dbg_q2", [128, NCOL], F32, kind="ExternalOutput")
        d_q3 = nc.dram_tensor("dbg_q3", [128, NCOL], F32, kind="ExternalOutput")

    strips = _strips()

    with tile.TileContext(nc) as tc, ExitStack() as ctx:
        wp = ctx.enter_context(tc.tile_pool(name="wp", bufs=1))
        big = ctx.enter_context(tc.tile_pool(name="big", bufs=1))
        xs = ctx.enter_context(tc.tile_pool(name="xs", bufs=2))
        sm = ctx.enter_context(tc.tile_pool(name="sm", bufs=3))
        fu = ctx.enter_context(tc.tile_pool(name="fu", bufs=2))
        pA = ctx.enter_context(tc.tile_pool(name="pA", bufs=2, space="PSUM"))
        pB = ctx.enter_context(tc.tile_pool(name="pB", bufs=2, space="PSUM"))
        pC = ctx.enter_context(tc.tile_pool(name="pC", bufs=2, space="PSUM"))
        pD = ctx.enter_context(tc.tile_pool(name="pD", bufs=2, space="PSUM"))

        # ---- weights / constants to SBUF ----
        w0c1 = wp.tile([128, 9 * 64], BF16)
        w0c2 = wp.tile([96, 3 * 64], BF16)
        w1P = wp.tile([128, 3 * 64], BF16)
        w1S = wp.tile([64, 3 * 64], BF16)
        w2P = wp.tile([128, 3 * 64], BF16)
        w2S = wp.tile([64, 3 * 64], BF16)
        wlP = wp.tile([128, 3 * 288], BF16)
        wlS = wp.tile([64, 3 * 288], BF16)
        bias = wp.tile([64, 6], F32)
        blp = wp.tile([128, 3], F32)
        eye = wp.tile([128, 32], BF16)
        for dst, src in ((w0c1, d_w0c1), (w0c2, d_w0c2), (w1P, d_w1P),
                         (w1S, d_w1S), (w2P, d_w2P), (w2S, d_w2S),
                         (wlP, d_wlP), (wlS, d_wlS), (eye, d_eye)):
            nc.sync.dma_start(out=dst[:], in_=src.ap())
        for dst, src in ((bias, d_b), (blp, d_blp)):
            nc.sync.dma_start(out=dst[:], in_=src.ap())

        def wslice(wt, i, m0, mw, step=64):
            # [K, mw] slice for matmul lhsT: tap/dc index i, out-ch offset m0
            return wt[:, i * step + m0: i * step + m0 + mw]

        def r_(t):
            return t

        # ---- big feature tiles (two slots: A holds q1 then q3, B holds q2) ----
        def new_q(tag):
            q = big.tile([128, NCOL], BF16, tag=tag)
            # zero the pad structure (lower half: head, inter-row cells, tail;
            # upper half: head cell + tail region never covered by upcopies)
            nc.gpsimd.memset(q[0:64, 0:130], 0.0)
            inter = q[0:64, 258:258 + 127 * ST].rearrange(
                "p (m s) -> p m s", s=ST)[:, :, 0:1]
            nc.gpsimd.memset(inter, 0.0)
            nc.gpsimd.memset(q[0:64, ST * 129:NCOL], 0.0)
            nc.gpsimd.memset(q[64:128, 0:1], 0.0)
            last_up = _j0(strips[-1][0]) - ST + strips[-1][1] * ST
            nc.gpsimd.memset(q[64:128, last_up:NCOL], 0.0)
            return q

        def evac_conv(ps, q, j0, nr, n, bcol):
            # leaky-relu from psum into q's valid cells + shifted upper copy
            rn = sm.tile([64, 3 * ST], F32, tag="rn")
            nc.scalar.activation(rn[:, 0:n], ps[:, 0:n], ACTF.Relu,
                                 bias=bias[:, bcol + 1:bcol + 2], scale=-0.99)
            src = ps[:, 0:n].rearrange("p (r c) -> p r c", c=ST)[:, :, 0:128]
            rnv = rn[:, 0:n].rearrange("p (r c) -> p r c", c=ST)[:, :, 0:128]
            dst = q[0:64, j0:j0 + n].rearrange("p (r c) -> p r c", c=ST)[:, :, 0:128]
            nc.vector.scalar_tensor_tensor(dst, src,
                                           bias[:, bcol:bcol + 1], rnv,
                                           op0=ALU.add, op1=ALU.add)
            # upper K-stack copy: up[j] = q[j+129] over this strip's window
            nc.sync.dma_start(out=q[64:128, j0 - ST:j0 - ST + n],
                              in_=q[0:64, j0:j0 + n])

        # ================= conv0 (streamed input strips) =================
        q1 = new_q("A")
        gi = 0
        while gi < len(strips):
            grp = strips[gi:gi + X_GROUP_STRIPS]
            r0g = grp[0][0]
            nrg = sum(nr for _, nr in grp)
            jg = _j0(r0g)
            win = ST * nrg + 260
            x1 = xs.tile([128, ST * 9 + 260], BF16, tag="x1")
            x2 = xs.tile([96, ST * 9 + 260], BF16, tag="x2")
            nc.sync.dma_start(out=x1[:, 0:win], in_=d_xpad.ap()[:, jg - 130:jg - 130 + win])
            for k in range(3):
                nc.sync.dma_start(
                    out=x2[32 * k:32 * k + 32, 0:win],
                    in_=d_mupad.ap()[:, jg - 130 + ST * k:jg - 130 + ST * k + win])
            for (r0, nr) in grp:
                j0 = _j0(r0)
                n = ST * nr
                loc = j0 - jg + 130
                ps = pA.tile([64, 3 * ST], F32, tag="pA")
                first = True
                for t, (dr, dc) in enumerate([(a, b) for a in (-1, 0, 1) for b in (-1, 0, 1)]):
                    o = loc + ST * dr + dc
                    nc.tensor.matmul(ps[:, 0:n], r_(wslice(w0c1, t, 0, 64)),
                                     r_(x1[:, o:o + n]), start=first, stop=False)
                    first = False
                for i, dc in enumerate((-1, 0, 1)):
                    o = loc - ST + dc
                    nc.tensor.matmul(ps[:, 0:n], r_(wslice(w0c2, i, 0, 64)),
                                     r_(x2[:, o:o + n]), start=False, stop=(i == 2))
                evac_conv(ps, q1, j0, nr, n, 0)
            gi += X_GROUP_STRIPS
        if debug:
            nc.sync.dma_start(out=d_q1.ap(), in_=q1[:])

        # ================= conv1 / conv2 =================
        def mid_conv(qin, qout, wP, wS, bcol):
            for (r0, nr) in strips:
                j0 = _j0(r0)
                n = ST * nr
                ps = pA.tile([64, 3 * ST], F32, tag="pA")
                for i, dc in enumerate((-1, 0, 1)):
                    o = j0 - ST + dc
                    nc.tensor.matmul(ps[:, 0:n], r_(wslice(wP, i, 0, 64)),
                                     r_(qin[0:128, o:o + n]), start=(i == 0), stop=False)
                for i, dc in enumerate((-1, 0, 1)):
                    o = j0 + ST + dc
                    nc.tensor.matmul(ps[:, 0:n], r_(wslice(wS, i, 0, 64)),
                                     r_(qin[0:64, o:o + n]), start=False, stop=(i == 2))
                evac_conv(ps, qout, j0, nr, n, bcol)

        q2 = new_q("B")
        mid_conv(q1, q2, w1P, w1S, 2)
        if debug:
            nc.sync.dma_start(out=d_q2.ap(), in_=q2[:])

        q3 = new_q("A")
        mid_conv(q2, q3, w2P, w2S, 4)
        if debug:
            nc.sync.dma_start(out=d_q3.ap(), in_=q3[:])

        # ================= conv_last + softmax + fusion =================
        for (r0, nr) in strips:
            j0 = _j0(r0)
            n = ST * nr
            ph = [pA.tile([128, 3 * ST], F32, tag="pA", name="ph0"),
                  pB.tile([128, 3 * ST], F32, tag="pB", name="ph1"),
                  pC.tile([32, 3 * ST], F32, tag="pC", name="ph2")]
            for ci, (m0, mw) in enumerate(((0, 128), (128, 128), (256, 32))):
                ps = ph[ci]
                for i, dc in enumerate((-1, 0, 1)):
                    o = j0 - ST + dc
                    nc.tensor.matmul(ps[:, 0:n], r_(wslice(wlP, i, m0, mw, 288)),
                                     r_(q3[0:128, o:o + n]), start=(i == 0), stop=False)
                for i, dc in enumerate((-1, 0, 1)):
                    o = j0 + ST + dc
                    nc.tensor.matmul(ps[:, 0:n], r_(wslice(wlS, i, m0, mw, 288)),
                                     r_(q3[0:64, o:o + n]), start=False, stop=(i == 2))
            # exp(head + b_last)
            ea = fu.tile([128, 3 * ST], BF16, tag="ea")
            eb = fu.tile([128, 3 * ST], BF16, tag="eb")
            ec = fu.tile([32, 3 * ST], BF16, tag="ec")
            nc.scalar.activation(ea[:, 0:n], ph[0][:, 0:n], ACTF.Exp, bias=blp[:, 0:1])
            nc.scalar.activation(eb[:, 0:n], ph[1][:, 0:n], ACTF.Exp, bias=blp[:, 1:2])
            nc.scalar.activation(ec[:, 0:n], ph[2][:, 0:n], ACTF.Exp, bias=blp[0:32, 2:3])
            # patch strips of mem_stab (xpad rows 96:128), z strip (rows 64:96)
            msa = fu.tile([128, 3 * ST], BF16, tag="msa")
            msb = fu.tile([128, 3 * ST], BF16, tag="msb")
            msc = fu.tile([32, 3 * ST], BF16, tag="msc")
            for g in range(4):
                nc.sync.dma_start(out=msa[32 * g:32 * g + 32, 0:n],
                                  in_=d_xpad.ap()[96:128, j0 + P_TAPS[g]:j0 + P_TAPS[g] + n])
                nc.sync.dma_start(out=msb[32 * g:32 * g + 32, 0:n],
                                  in_=d_xpad.ap()[96:128, j0 + P_TAPS[4 + g]:j0 + P_TAPS[4 + g] + n])
            nc.sync.dma_start(out=msc[:, 0:n],
                              in_=d_xpad.ap()[96:128, j0 + P_TAPS[8]:j0 + P_TAPS[8] + n])
            rhs3 = fu.tile([64, 3 * ST], BF16, tag="rhs3")
            nc.sync.dma_start(out=rhs3[32:64, 0:n], in_=d_xpad.ap()[64:96, j0:j0 + n])
            ta = fu.tile([128, 3 * ST], BF16, tag="ta")
            tb = fu.tile([128, 3 * ST], BF16, tag="tb")
            nc.vector.tensor_mul(ta[:, 0:n], ea[:, 0:n], msa[:, 0:n])
            nc.vector.tensor_mul(tb[:, 0:n], eb[:, 0:n], msb[:, 0:n])
            nc.vector.tensor_mul(rhs3[0:32, 0:n], ec[:, 0:n], msc[:, 0:n])
            # numerator (psum 0:32) and denominator (psum 32:64)
            nd = pD.tile([64, 3 * ST], F32, tag="pD")
            nc.tensor.matmul(nd[0:32, 0:n], r_(eye[:]), r_(ta[:, 0:n]), start=True, stop=False)
            nc.tensor.matmul(nd[0:32, 0:n], r_(eye[:]), r_(tb[:, 0:n]), start=False, stop=False)
            nc.tensor.matmul(nd[0:32, 0:n], r_(eye[0:64, :]), r_(rhs3[:, 0:n]), start=False, stop=True)
            nc.tensor.matmul(nd[32:64, 0:n], r_(eye[:]), r_(ea[:, 0:n]), start=True, stop=False)
            nc.tensor.matmul(nd[32:64, 0:n], r_(eye[:]), r_(eb[:, 0:n]), start=False, stop=False)
            nc.tensor.matmul(nd[32:64, 0:n], r_(eye[0:32, :]), r_(ec[:, 0:n]), start=False, stop=True)
            den = fu.tile([32, 3 * ST], F32, tag="den")
            rde = fu.tile([32, 3 * ST], F32, tag="rde")
            ost = fu.tile([32, 3 * ST], F32, tag="ost")
            nc.vector.tensor_scalar_add(den[:, 0:n], nd[32:64, 0:n], 1.0)
            nc.vector.reciprocal_approx_fast(rde[:, 0:n], den[:, 0:n])
            nc.vector.tensor_mul(ost[:, 0:n], nd[0:32, 0:n], rde[:, 0:n])
            src = ost[:, 0:n].rearrange("p (r c) -> p r c", c=ST)[:, :, 0:128]
            nc.sync.dma_start(out=d_out.ap()[:, r0:r0 + nr, :], in_=src)

    nc.compile()
    return nc


BF16_NP = mybir.dt.np(mybir.dt.bfloat16)


def _pad_rows(x, cols):
    # x: [C, 128, 128] -> zero-padded flat rows [C, cols], bf16
    c = x.shape[0]
    buf = np.zeros((c, cols), dtype=BF16_NP)
    buf[:, 130:130 + ST * 128].reshape(c, 128, ST)[:, :, 0:128] = x.astype(BF16_NP)
    return buf


def _prep_shared(w0, b0, w1, b1, w2, b2, w_last, b_last):
    f = np.float32
    w0t = np.transpose(np.asarray(w0, f), (1, 2, 3, 0))      # [160,3,3,64]
    w0c1 = np.ascontiguousarray(w0t[0:128].reshape(128, 9 * 64))
    w0c2 = np.ascontiguousarray(
        np.transpose(w0t[128:160], (1, 0, 2, 3)).reshape(96, 3 * 64))
    def mid(w):
        wt = np.transpose(np.asarray(w, f), (1, 2, 3, 0))    # [64,3,3,64]
        wP = np.ascontiguousarray(
            np.concatenate([wt[:, 0], wt[:, 1]], 0).reshape(128, 3 * 64))
        wS = np.ascontiguousarray(wt[:, 2].reshape(64, 3 * 64))
        return wP, wS
    w1P, w1S = mid(w1)
    w2P, w2S = mid(w2)
    perm = np.array([(pp % 32) * 9 + pp // 32 for pp in range(288)])
    wl2 = np.asarray(w_last, f)[perm]                        # [288,64,3,3] p-major
    wlt = np.transpose(wl2, (1, 2, 3, 0))                    # [64,3,3,288]
    wlP = np.ascontiguousarray(
        np.concatenate([wlt[:, 0], wlt[:, 1]], 0).reshape(128, 3 * 288))
    wlS = np.ascontiguousarray(wlt[:, 2].reshape(64, 3 * 288))
    bias = np.stack([np.asarray(b0, f), -0.99 * np.asarray(b0, f),
                     np.asarray(b1, f), -0.99 * np.asarray(b1, f),
                     np.asarray(b2, f), -0.99 * np.asarray(b2, f)], axis=1)
    blp_flat = np.asarray(b_last, f)[perm]
    blp = np.zeros((128, 3), f)
    blp[:, 0] = blp_flat[0:128]
    blp[:, 1] = blp_flat[128:256]
    blp[0:32, 2] = blp_flat[256:288]
    eye = np.tile(np.eye(32, dtype=f), (4, 1))
    out = dict(w0c1=w0c1, w0c2=w0c2, w1P=w1P, w1S=w1S, w2P=w2P, w2S=w2S,
               wlP=wlP, wlS=wlS, eye=eye)
    out = {k: v.astype(BF16_NP) for k, v in out.items()}
    out["bias"] = np.ascontiguousarray(bias)
    out["blp"] = blp
    return out


_NC_CACHE = {}


def _get_nc(debug=False):
    if debug not in _NC_CACHE:
        _NC_CACHE[debug] = _build_program(debug)
    return _NC_CACHE[debug]


def make_in_maps(z, backbone, mem_stab, mem_unstab, shared):
    f = np.float32
    z = np.asarray(z, f); backbone = np.asarray(backbone, f)
    ms = np.asarray(mem_stab, f); mu = np.asarray(mem_unstab, f)
    maps = []
    for b in range(z.shape[0]):
        x160 = np.concatenate([backbone[b], z[b], ms[b]], axis=0)  # [128,...]
        maps.append(dict(xpad=_pad_rows(x160, XCOL),
                         mupad=_pad_rows(mu[b], MUCOL), **shared))
    return maps


def kernel(z, backbone, mem_stab, mem_unstab, w0, b0, w1, b1, w2, b2,
           w_last, b_last, fusion_kernel_size):
    assert int(fusion_kernel_size) == 3
    shared = _prep_shared(w0, b0, w1, b1, w2, b2, w_last, b_last)
    in_maps = make_in_maps(z, backbone, mem_stab, mem_unstab, shared)
    nc = _get_nc()
    res = run_bass_kernel_spmd(nc, in_maps, core_ids=list(range(len(in_maps))))
    out = np.stack([r["out"] for r in res.results], axis=0)
    return out.astype(np.float32)



# revision 2
# speedup vs baseline: 1.0079x; 1.0079x over previous
"""Trainium2 Bass kernel for nn_ControlledConvEMAStabilizer (v4).

4-row strips, N=512 matmuls (2D-strided rhs skips the shared pad column;
flat [*,512] psum = exactly one bank). All 3x3 convs are 5 matmuls:
  3 mm {kh0,kh1}xdc (K=128 vertical stack) + 1 mm {kh2,-1|kh2,0} (K=128 via
  the horizontally-stacked hs tile) + 1 mm {kh2,+1} zero-padded to K=128.
Fusion softmax reduction is 5 matmuls (combined num/den eye for the
tc/z/ec tail, K=96). Every conv matmul has K=128.

Deep software pipeline, one emission loop (keeps the PE HAM-warm):
  t: patches+conv_last+exp+mul(t-9), conv0(t), conv1(t-3), conv2(t-6), nd(t-10)
"""

import numpy as np
from contextlib import ExitStack

import concourse.bacc as bacc
import concourse.tile as tile
from concourse import mybir
from concourse.bass_utils import run_bass_kernel_spmd

F32 = mybir.dt.float32
BF16 = mybir.dt.bfloat16
ALU = mybir.AluOpType
ACTF = mybir.ActivationFunctionType

H = 128
ST = 129
NCOL = ST * 130 + 2
XCOL = NCOL
MUCOL = NCOL + 2 * ST + 2
RPS = 4
NPIX = RPS * 128              # 512
NWIN = RPS * ST               # 516
XG = 2                        # strips per conv0 input group

P_TAPS = [ST * (kh - 1) + (kw - 1) for kh in range(3) for kw in range(3)]

L1, L2, L3, L4 = 3, 6, 9, 10  # stage lags


def _j0(r0):
    return ST * (r0 + 1) + 1


def _build_program():
    nc = bacc.Bacc("TRN2", target_bir_lowering=False, debug=False)

    d_xpad = nc.dram_tensor("xpad", [128, XCOL], BF16, kind="ExternalInput")
    d_mupad = nc.dram_tensor("mupad", [32, MUCOL], BF16, kind="ExternalInput")
    d_w0c1 = nc.dram_tensor("w0c1", [128, 9 * 64], BF16, kind="ExternalInput")
    d_w0c2 = nc.dram_tensor("w0c2", [128, 3 * 64], BF16, kind="ExternalInput")
    d_w1P = nc.dram_tensor("w1P", [128, 3 * 64], BF16, kind="ExternalInput")
    d_w1H = nc.dram_tensor("w1H", [128, 64], BF16, kind="ExternalInput")
    d_w1S = nc.dram_tensor("w1S", [128, 64], BF16, kind="ExternalInput")
    d_w2P = nc.dram_tensor("w2P", [128, 3 * 64], BF16, kind="ExternalInput")
    d_w2H = nc.dram_tensor("w2H", [128, 64], BF16, kind="ExternalInput")
    d_w2S = nc.dram_tensor("w2S", [128, 64], BF16, kind="ExternalInput")
    d_wlP = nc.dram_tensor("wlP", [128, 3 * 288], BF16, kind="ExternalInput")
    d_wlH = nc.dram_tensor("wlH", [128, 288], BF16, kind="ExternalInput")
    d_wlS = nc.dram_tensor("wlS", [128, 288], BF16, kind="ExternalInput")
    d_b = nc.dram_tensor("bias", [64, 6], F32, kind="ExternalInput")
    d_blp = nc.dram_tensor("blp", [128, 3], F32, kind="ExternalInput")
    d_eye = nc.dram_tensor("eye", [128, 32], BF16, kind="ExternalInput")
    d_eyeC = nc.dram_tensor("eyeC", [96, 64], BF16, kind="ExternalInput")
    d_out = nc.dram_tensor("out", [32, H, H], F32, kind="ExternalOutput")

    strips = [(r0, RPS) for r0 in range(0, H, RPS)]
    S = len(strips)

    def pix(t, parts, o):
        return t[parts[0]:parts[1], o:o + NWIN].rearrange(
            "p (r c) -> p r c", c=ST)[:, :, 0:128]

    with tile.TileContext(nc) as tc, ExitStack() as ctx:
        wp = ctx.enter_context(tc.tile_pool(name="wp", bufs=1))
        big = ctx.enter_context(tc.tile_pool(name="big", bufs=1))
        xs = ctx.enter_context(tc.tile_pool(name="xs", bufs=2))
        sm = ctx.enter_context(tc.tile_pool(name="sm", bufs=3))
        fu = ctx.enter_context(tc.tile_pool(name="fu", bufs=3))
        hsp = ctx.enter_context(tc.tile_pool(name="hsp", bufs=4))
        pcv = ctx.enter_context(tc.tile_pool(name="pcv", bufs=3, space="PSUM"))
        pch = ctx.enter_context(tc.tile_pool(name="pch", bufs=3, space="PSUM"))
        pnd = ctx.enter_context(tc.tile_pool(name="pnd", bufs=2, space="PSUM"))

        w0c1 = wp.tile([128, 9 * 64], BF16)
        w0c2 = wp.tile([128, 3 * 64], BF16)
        w1P = wp.tile([128, 3 * 64], BF16)
        w1H = wp.tile([128, 64], BF16)
        w1S = wp.tile([128, 64], BF16)
        w2P = wp.tile([128, 3 * 64], BF16)
        w2H = wp.tile([128, 64], BF16)
        w2S = wp.tile([128, 64], BF16)
        wlP = wp.tile([128, 3 * 288], BF16)
        wlH = wp.tile([128, 288], BF16)
        wlS = wp.tile([128, 288], BF16)
        bias = wp.tile([64, 6], F32)
        blp = wp.tile([128, 3], F32)
        eye = wp.tile([128, 32], BF16)
        eyeC = wp.tile([96, 64], BF16)
        def load_weights():
            for dst, src in ((w0c1, d_w0c1), (w0c2, d_w0c2), (bias, d_b),
                             (w1P, d_w1P), (w1H, d_w1H), (w1S, d_w1S),
                             (w2P, d_w2P), (w2H, d_w2H), (w2S, d_w2S),
                             (wlP, d_wlP), (wlH, d_wlH), (wlS, d_wlS),
                             (eye, d_eye), (eyeC, d_eyeC), (blp, d_blp)):
                nc.scalar.dma_start(out=dst[:], in_=src.ap())

        def wslice(wt, i, m0, mw, step=64):
            return wt[:, i * step + m0: i * step + m0 + mw]

        def new_q(tag):
            q = big.tile([128, NCOL], BF16, tag=tag, name="q_" + tag)
            nc.gpsimd.memset(q[0:64, 0:130], 0.0)
            inter = q[0:64, 258:258 + 127 * ST].rearrange(
                "p (m s) -> p m s", s=ST)[:, :, 0:1]
            nc.gpsimd.memset(inter, 0.0)
            nc.gpsimd.memset(q[0:64, ST * 129:NCOL], 0.0)
            nc.gpsimd.memset(q[64:128, 0:1], 0.0)
            last_up = _j0(strips[-1][0]) - ST + NWIN
            nc.gpsimd.memset(q[64:128, last_up:NCOL], 0.0)
            return q

        q1 = new_q("A")
        q2 = new_q("B")
        q3 = new_q("C")

        def evac_conv(ps, q, j0, bcol):
            rn = sm.tile([64, NPIX], F32, tag="rn", name="rn")
            nc.scalar.activation(rn[:], ps[:, 0:NPIX], ACTF.Relu,
                                 bias=bias[:, bcol + 1:bcol + 2], scale=-0.99)
            src = ps[:, 0:NPIX].rearrange("p (r c) -> p r c", c=128)
            rnv = rn[:].rearrange("p (r c) -> p r c", c=128)
            dst = pix(q, (0, 64), j0)
            nc.vector.scalar_tensor_tensor(dst, src,
                                           bias[:, bcol:bcol + 1], rnv,
                                           op0=ALU.add, op1=ALU.add)
            nc.gpsimd.dma_start(out=q[64:128, j0 - ST:j0 - ST + NWIN],
                                in_=q[0:64, j0:j0 + NWIN])

        def build_hs(q, i, htag):
            r0, _ = strips[i]
            j0 = _j0(r0)
            hs = hsp.tile([128, NWIN], BF16, tag=htag, name="hs_" + htag)
            nc.gpsimd.dma_start(out=hs[0:64, 0:NWIN],
                                in_=q[0:64, j0 + ST - 1:j0 + ST - 1 + NWIN])
            nc.gpsimd.dma_start(out=hs[64:128, 0:NWIN],
                                in_=q[0:64, j0 + ST:j0 + ST + NWIN])
            return hs

        hs_stash = {}

        def conv5(dst_ps, qin, hs, j0, wP, wH, wS, m0=0, mw=64, step=64):
            # 5-matmul 3x3 conv over one 4-row strip; all K=128
            for k, dc in enumerate((-1, 0, 1)):
                nc.tensor.matmul(dst_ps, wslice(wP, k, m0, mw, step),
                                 pix(qin, (0, 128), j0 - ST + dc),
                                 start=(k == 0), stop=False)
            nc.tensor.matmul(dst_ps, wH[:, m0:m0 + mw],
                             pix(hs, (0, 128), 0), start=False, stop=False)
            # {kh2,+1}: weights rows 64:128 are zero; upper half of qin is
            # the +129-shifted copy (always defined), contributes 0.
            nc.tensor.matmul(dst_ps, wS[:, m0:m0 + mw],
                             pix(qin, (0, 128), j0 + ST + 1),
                             start=False, stop=True)

        def conv0_strip(i, xg):
            x1, x2, jg = xg
            r0, _ = strips[i]
            j0 = _j0(r0)
            loc = j0 - jg + 130
            ps = pcv.tile([64, NPIX], F32, tag="cv", name="ps0")
            first = True
            for t, (dr, dc) in enumerate([(a, b) for a in (-1, 0, 1) for b in (-1, 0, 1)]):
                o = loc + ST * dr + dc
                nc.tensor.matmul(ps[:], wslice(w0c1, t, 0, 64),
                                 pix(x1, (0, 128), o), start=first, stop=False)
                first = False
            for k, dc in enumerate((-1, 0, 1)):
                o = loc - ST + dc
                nc.tensor.matmul(ps[:], wslice(w0c2, k, 0, 64),
                                 pix(x2, (0, 128), o), start=False, stop=(k == 2))
            evac_conv(ps, q1, j0, 0)

        def mid_conv_strip(i, qin, qout, wP, wH, wS, bcol, htag):
            r0, _ = strips[i]
            j0 = _j0(r0)
            ps = pcv.tile([64, NPIX], F32, tag="cv", name="psm")
            conv5(ps[:], qin, hs_stash[(htag, i)], j0, wP, wH, wS)
            evac_conv(ps, qout, j0, bcol)

        def pixd(o, parts):
            return d_xpad.ap()[parts[0]:parts[1], o:o + NWIN].rearrange(
                "p (r c) -> p r c", c=ST)[:, :, 0:128]

        def tail_head_strip(i):
            r0, _ = strips[i]
            j0 = _j0(r0)
            hs = hs_stash[("h3", i)]
            msa = fu.tile([128, NPIX], BF16, tag="msa", name="msa")
            msb = fu.tile([128, NPIX], BF16, tag="msb", name="msb")
            msc = fu.tile([32, NPIX], BF16, tag="msc", name="msc")
            rhs3 = fu.tile([96, NPIX], BF16, tag="rhs3", name="rhs3")
            for g in range(4):
                nc.sync.dma_start(
                    out=msa[32 * g:32 * g + 32, 0:NPIX].rearrange("p (r c) -> p r c", c=128),
                    in_=pixd(j0 + P_TAPS[g], (96, 128)))
                nc.sync.dma_start(
                    out=msb[32 * g:32 * g + 32, 0:NPIX].rearrange("p (r c) -> p r c", c=128),
                    in_=pixd(j0 + P_TAPS[4 + g], (96, 128)))
            nc.gpsimd.dma_start(
                out=msc[:, 0:NPIX].rearrange("p (r c) -> p r c", c=128),
                in_=pixd(j0 + P_TAPS[8], (96, 128)))
            nc.gpsimd.dma_start(
                out=rhs3[32:64, 0:NPIX].rearrange("p (r c) -> p r c", c=128),
                in_=pixd(j0, (64, 96)))

            ps1 = pch.tile([128, NPIX], F32, tag="ch", name="ph1")
            ps2 = pch.tile([128, NPIX], F32, tag="ch", name="ph2")
            ps3 = pch.tile([32, NPIX], F32, tag="ch", name="ph3")
            for (pst, p0), (m0, mw) in zip(((ps1, 0), (ps2, 0), (ps3, 0)),
                                           ((0, 128), (128, 128), (256, 32))):
                conv5(pst[p0:p0 + mw, 0:NPIX], q3, hs, j0, wlP, wlH, wlS,
                      m0=m0, mw=mw, step=288)
            ea = fu.tile([128, NPIX], BF16, tag="ea", name="ea")
            eb = fu.tile([128, NPIX], BF16, tag="eb", name="eb")
            ec = fu.tile([32, NPIX], BF16, tag="ec", name="ec")
            nc.scalar.activation(ea[:], ps1[:, 0:NPIX], ACTF.Exp, bias=blp[:, 0:1])
            nc.scalar.activation(eb[:], ps2[:, 0:NPIX], ACTF.Exp, bias=blp[:, 1:2])
            nc.scalar.activation(ec[:], ps3[0:32, 0:NPIX],
                                 ACTF.Exp, bias=blp[0:32, 2:3])
            nc.gpsimd.dma_start(out=rhs3[64:96, 0:NPIX], in_=ec[:])
            ta = fu.tile([128, NPIX], BF16, tag="ta", name="ta")
            tb = fu.tile([128, NPIX], BF16, tag="tb", name="tb")
            nc.vector.tensor_mul(ta[:], ea[:], msa[:])
            nc.vector.tensor_mul(tb[:], eb[:], msb[:])
            nc.vector.tensor_mul(rhs3[0:32, 0:NPIX], ec[:], msc[:])
            return (ea, eb, ta, tb, rhs3)

        tail_stash = {}

        def tail_nd_strip(i):
            r0, _ = strips[i]
            ea, eb, ta, tb, rhs3 = tail_stash.pop(i)
            nd = pnd.tile([64, NPIX], F32, tag="nd", name="nd")
            nc.tensor.matmul(nd[0:32, 0:NPIX], eye[:], ta[:], start=True, stop=False)
            nc.tensor.matmul(nd[0:32, 0:NPIX], eye[:], tb[:], start=False, stop=False)
            nc.tensor.matmul(nd[32:64, 0:NPIX], eye[:], ea[:], start=True, stop=False)
            nc.tensor.matmul(nd[32:64, 0:NPIX], eye[:], eb[:], start=False, stop=False)
            # combined: num += tc + z (cols 0:32), den += ec (cols 32:64)
            nc.tensor.matmul(nd[0:64, 0:NPIX], eyeC[:], rhs3[0:96, 0:NPIX],
                             start=False, stop=True, skip_group_check=True)
            den = fu.tile([32, NPIX], F32, tag="den", name="den")
            rde = fu.tile([32, NPIX], F32, tag="rde", name="rde")
            ost = fu.tile([32, NPIX], F32, tag="ost", name="ost")
            nc.vector.tensor_scalar_add(den[:], nd[32:64, 0:NPIX], 1.0)
            nc.vector.reciprocal_approx_fast(rde[:], den[:])
            nc.vector.tensor_mul(ost[:], nd[0:32, 0:NPIX], rde[:])
            nc.sync.dma_start(out=d_out.ap()[:, r0:r0 + RPS, :],
                              in_=ost[:].rearrange("p (r c) -> p r c", c=128))

        def load_xgroup(gi):
            grp = strips[gi:gi + XG]
            r0g = grp[0][0]
            nrg = sum(nr for _, nr in grp)
            jg = _j0(r0g)
            win = ST * nrg + 260
            x1 = xs.tile([128, ST * 2 * RPS + 260], BF16, tag="x1", name="x1")
            x2 = xs.tile([128, ST * 2 * RPS + 260], BF16, tag="x2", name="x2")
            nc.sync.dma_start(out=x1[:, 0:win], in_=d_xpad.ap()[:, jg - 130:jg - 130 + win])
            for k in range(3):
                (nc.gpsimd if gi == 0 else nc.sync).dma_start(
                    out=x2[32 * k:32 * k + 32, 0:win],
                    in_=d_mupad.ap()[:, jg - 130 + ST * k:jg - 130 + ST * k + win])
            nc.gpsimd.memset(x2[96:128, 0:win], 0.0)
            return (x1, x2, jg)

        # ---- the interleaved pipeline ----
        xgroups = {0: load_xgroup(0)}
        load_weights()
        for t in range(S + L4):
            if t % XG == 1 and (t + 1) < S:
                xgroups[(t + 1) // XG] = load_xgroup(t + 1)
            if L3 <= t < S + L3:
                i = t - L3
                tail_stash[i] = tail_head_strip(i)
                hs_stash.pop(("h3", i))
            if t < S:
                conv0_strip(t, xgroups[t // XG])
                if t % XG == XG - 1 or t == S - 1:
                    xgroups.pop(t // XG, None)
                if t >= 1:
                    hs_stash[("h1", t - 1)] = build_hs(q1, t - 1, "h1")
                if t == S - 1:
                    hs_stash[("h1", t)] = build_hs(q1, t, "h1")
            if L1 <= t < S + L1:
                i = t - L1
                mid_conv_strip(i, q1, q2, w1P, w1H, w1S, 2, "h1")
                hs_stash.pop(("h1", i))
                if i >= 1:
                    hs_stash[("h2", i - 1)] = build_hs(q2, i - 1, "h2")
                if i == S - 1:
                    hs_stash[("h2", i)] = build_hs(q2, i, "h2")
            if L2 <= t < S + L2:
                i = t - L2
                mid_conv_strip(i, q2, q3, w2P, w2H, w2S, 4, "h2")
                hs_stash.pop(("h2", i))
                if i >= 1:
                    hs_stash[("h3", i - 1)] = build_hs(q3, i - 1, "h3")
                if i == S - 1:
                    hs_stash[("h3", i)] = build_hs(q3, i, "h3")
            if L4 <= t < S + L4:
                tail_nd_strip(t - L4)

    nc.compile()
    return nc


BF16_NP = mybir.dt.np(mybir.dt.bfloat16)


def _pad_rows(x, cols):
    c = x.shape[0]
    buf = np.zeros((c, cols), dtype=BF16_NP)
    buf[:, 130:130 + ST * 128].reshape(c, 128, ST)[:, :, 0:128] = x.astype(BF16_NP)
    return buf


def _zpad128(a):
    # pad partition dim (rows) with zeros to 128
    out = np.zeros((128, a.shape[1]), a.dtype)
    out[0:a.shape[0]] = a
    return out


def _prep_shared(w0, b0, w1, b1, w2, b2, w_last, b_last):
    f = np.float32
    w0t = np.transpose(np.asarray(w0, f), (1, 2, 3, 0))      # [160,3,3,64]
    w0c1 = np.ascontiguousarray(w0t[0:128].reshape(128, 9 * 64))
    w0c2 = _zpad128(np.ascontiguousarray(
        np.transpose(w0t[128:160], (1, 0, 2, 3)).reshape(96, 3 * 64)))

    def c5(wt, M):
        # 5-mm layouts: wP [128, 3*M] ({kh0,kh1} x dc), wH [128, M]
        # ({kh2,kw0};{kh2,kw1}), wS [128, M] ({kh2,kw2} zero-padded)
        wP = np.ascontiguousarray(
            np.concatenate([wt[:, 0], wt[:, 1]], 0).reshape(128, 3 * M))
        wH = np.ascontiguousarray(
            np.concatenate([wt[:, 2, 0], wt[:, 2, 1]], 0))   # [128, M]
        wS = _zpad128(np.ascontiguousarray(wt[:, 2, 2]))     # [128, M]
        return wP, wH, wS

    w1P, w1H, w1S = c5(np.transpose(np.asarray(w1, f), (1, 2, 3, 0)), 64)
    w2P, w2H, w2S = c5(np.transpose(np.asarray(w2, f), (1, 2, 3, 0)), 64)
    perm = np.array([(pp % 32) * 9 + pp // 32 for pp in range(288)])
    wl2 = np.asarray(w_last, f)[perm]
    wlP, wlH, wlS = c5(np.transpose(wl2, (1, 2, 3, 0)), 288)
    bias = np.stack([np.asarray(b0, f), -0.99 * np.asarray(b0, f),
                     np.asarray(b1, f), -0.99 * np.asarray(b1, f),
                     np.asarray(b2, f), -0.99 * np.asarray(b2, f)], axis=1)
    blp_flat = np.asarray(b_last, f)[perm]
    blp = np.zeros((128, 3), f)
    blp[:, 0] = blp_flat[0:128]
    blp[:, 1] = blp_flat[128:256]
    blp[0:32, 2] = blp_flat[256:288]
    eye = np.tile(np.eye(32, dtype=f), (4, 1))               # [128, 32]
    eyeC = np.zeros((96, 64), f)                             # [tc; z; ec]
    eyeC[0:32, 0:32] = np.eye(32)
    eyeC[32:64, 0:32] = np.eye(32)
    eyeC[64:96, 32:64] = np.eye(32)
    out = dict(w0c1=w0c1, w0c2=w0c2, w1P=w1P, w1H=w1H, w1S=w1S,
               w2P=w2P, w2H=w2H, w2S=w2S, wlP=wlP, wlH=wlH, wlS=wlS,
               eye=eye, eyeC=eyeC)
    out = {k: v.astype(BF16_NP) for k, v in out.items()}
    out["bias"] = np.ascontiguousarray(bias)
    out["blp"] = blp
    return out


_NC_CACHE = {}


def _get_nc(m64_last=False):
    if "nc" not in _NC_CACHE:
        _NC_CACHE["nc"] = _build_program()
    return _NC_CACHE["nc"]


def make_in_maps(z, backbone, mem_stab, mem_unstab, shared):
    f = np.float32
    z = np.asarray(z, f); backbone = np.asarray(backbone, f)
    ms = np.asarray(mem_stab, f); mu = np.asarray(mem_unstab, f)
    maps = []
    for b in range(z.shape[0]):
        x160 = np.concatenate([backbone[b], z[b], ms[b]], axis=0)
        maps.append(dict(xpad=_pad_rows(x160, XCOL),
                         mupad=_pad_rows(mu[b], MUCOL), **shared))
    return maps


def kernel(z, backbone, mem_stab, mem_unstab, w0, b0, w1, b1, w2, b2,
           w_last, b_last, fusion_kernel_size):
    assert int(fusion_kernel_size) == 3
    shared = _prep_shared(w0, b0, w1, b1, w2, b2, w_last, b_last)
    in_maps = make_in_maps(z, backbone, mem_stab, mem_unstab, shared)
    nc = _get_nc()
    res = run_bass_kernel_spmd(nc, in_maps, core_ids=list(range(len(in_maps))))
    out = np.stack([r["out"] for r in res.results], axis=0)
    return out.astype(np.float32)
